# revision 2
# baseline (speedup 1.0000x reference)
"""Bass/Trainium2 kernel for nn_BespokeEmbedding (moe_routing).

Strategy (data-parallel over tokens across 8 NeuronCores):
  - Host computes per-token category codes (cat_table[token_ids]) and routes
    each core's 4096 tokens into 4 per-category groups (the "dispatch" step of
    the expert routing), gathering each group's embedding rows into a
    contraction-major [d_c, M_PAD] activation block per category.
  - Each core runs one Bass/Tile kernel: for every category, a dense
    float32r matmul  Y_c^T = W_c^T @ X_c^T  accumulated over 128-row K tiles
    in PSUM, fused bias-add on the Scalar engine, result streamed back as
    Y_c^T [D, M_PAD].
  - Host scatters rows back to token order (inverse of the dispatch) and
    returns the full [8, 4096, 1024] float32 output.

float32r keeps the full fp32 input bits and runs the PE at 1 cycle/row
(vs 4 for strict fp32); measured matmul relative error ~1.5e-4 at K=1536.
"""

import numpy as np

B, S, V, D = 8, 4096, 50257, 1024
CAT_DIMS = (1536, 1024, 512, 256)
NAMES = ("high", "mid", "low", "special")
N_CORES = 8
TOK_PER_CORE = (B * S) // N_CORES      # 4096
M_PAD = 1280                           # per-core per-category padded group size
CHUNKS = ((0, 512), (512, 512), (1024, 256))   # token chunks of M_PAD
N_DCOL = D // 128                      # 8

_CACHE = {}
LAST_EXEC_NS = None
LAST_RESULTS = None


def _build_bass():
    from contextlib import ExitStack
    import concourse.bacc as bacc
    import concourse.mybir as mybir
    import concourse.tile as tile

    nc = bacc.Bacc("TRN2", target_bir_lowering=False, debug=False,
                   num_devices=N_CORES)
    f32r = mybir.dt.float32r
    f32 = mybir.dt.float32

    xt_d, w_d, yt_d = {}, {}, {}
    for nm, d in zip(NAMES, CAT_DIMS):
        xt_d[nm] = nc.dram_tensor(f"xt_{nm}", [d, M_PAD], f32r, kind="ExternalInput")
        w_d[nm] = nc.dram_tensor(f"w_{nm}", [d, D], f32r, kind="ExternalInput")
        yt_d[nm] = nc.dram_tensor(f"yt_{nm}", [D, M_PAD], f32, kind="ExternalOutput")
    # bias packed host-side as [128, 4*8]: column c*8+j holds b_c[j*128:(j+1)*128]
    bias_d = nc.dram_tensor("bias", [128, len(NAMES) * N_DCOL], f32,
                            kind="ExternalInput")

    with tile.TileContext(nc) as tc, ExitStack() as ctx:
        wpool = ctx.enter_context(tc.tile_pool(name="w", bufs=1))
        xpool = ctx.enter_context(tc.tile_pool(name="x", bufs=2))
        opool = ctx.enter_context(tc.tile_pool(name="o", bufs=2))
        bpool = ctx.enter_context(tc.tile_pool(name="b", bufs=1))
        ppool = ctx.enter_context(tc.tile_pool(name="p", bufs=4, space="PSUM"))

        bias_t = bpool.tile([128, len(NAMES) * N_DCOL], f32)
        nc.sync.dma_start(bias_t[:], bias_d.ap())

        # all four weight matrices stay resident in SBUF for the whole kernel:
        # total sum(d_c)/128 * 1024 fp32 = 104 KiB/partition
        w_t = {}
        for nm, d in zip(NAMES, CAT_DIMS):
            nk = d // 128
            w_t[nm] = wpool.tile([128, nk * D], f32r, tag=f"w_{nm}", name=f"w_{nm}_sb")
            nc.sync.dma_start(
                w_t[nm][:].rearrange("p (k m) -> p k m", k=nk),
                w_d[nm].ap().rearrange("(k p) m -> p k m", p=128),
            )

        for ci, (nm, d) in enumerate(zip(NAMES, CAT_DIMS)):
            nk = d // 128
            for c0, n in CHUNKS:
                # activation slab [d, n] for this token chunk, K-major layout
                x_t = xpool.tile([128, 12 * 512], f32r, tag="xslab")
                nc.sync.dma_start(
                    x_t[:, :nk * n].rearrange("p (k t) -> p k t", k=nk),
                    xt_d[nm].ap()[:, c0:c0 + n].rearrange("(k p) t -> p k t", p=128),
                )
                o_t = opool.tile([128, N_DCOL * 512], f32, tag="ostage")
                for j in range(N_DCOL):
                    ps = ppool.tile([128, 512], f32, tag="acc")
                    for k in range(nk):
                        nc.tensor.matmul(
                            ps[:, :n],
                            w_t[nm][:, k * D + j * 128: k * D + (j + 1) * 128],
                            x_t[:, k * n: (k + 1) * n],
                            start=(k == 0),
                            stop=(k == nk - 1),
                        )
                    nc.scalar.activation(
                        o_t[:, j * n: (j + 1) * n],
                        ps[:, :n],
                        bass_ident(),
                        bias=bias_t[:, ci * N_DCOL + j: ci * N_DCOL + j + 1],
                    )
                nc.sync.dma_start(
                    yt_d[nm].ap()[:, c0:c0 + n].rearrange("(j p) t -> p j t", p=128),
                    o_t[:, :N_DCOL * n].rearrange("p (j t) -> p j t", j=N_DCOL),
                )
    nc.compile()
    return nc


def bass_ident():
    import concourse.mybir as mybir
    return mybir.ActivationFunctionType.Identity


def _get_nc():
    if "nc" not in _CACHE:
        _CACHE["nc"] = _build_bass()
    return _CACHE["nc"]


def kernel(_profile=False, **inputs):
    global LAST_EXEC_NS, LAST_RESULTS
    from concourse.bass_utils import run_bass_kernel_spmd

    token_ids = np.asarray(inputs["token_ids"]).astype(np.int64)
    cat_table = np.asarray(inputs["cat_table"]).astype(np.int64)
    emb = {nm: np.ascontiguousarray(np.asarray(inputs[f"emb_{nm}"], dtype=np.float32))
           for nm in NAMES}
    W = {nm: np.ascontiguousarray(np.asarray(inputs[f"W_{nm}"], dtype=np.float32))
         for nm in NAMES}
    bvec = {nm: np.asarray(inputs[f"b_{nm}"], dtype=np.float32) for nm in NAMES}

    # bias packed as [128, 4*8]
    bias_packed = np.concatenate(
        [bvec[nm].reshape(N_DCOL, 128).T for nm in NAMES], axis=1
    )
    bias_packed = np.ascontiguousarray(bias_packed, dtype=np.float32)

    tok_flat = token_ids.reshape(-1)          # [32768]
    cats = cat_table[tok_flat]                # [32768]

    in_maps = []
    route = []      # per core: {nm: (positions, n_used)}
    overflow = []   # (core, nm, positions_beyond_cap)
    for core in range(N_CORES):
        lo = core * TOK_PER_CORE
        t = tok_flat[lo:lo + TOK_PER_CORE]
        c = cats[lo:lo + TOK_PER_CORE]
        im = {"bias": bias_packed}
        r = {}
        for ci, (nm, d) in enumerate(zip(NAMES, CAT_DIMS)):
            pos = np.nonzero(c == ci)[0]
            n = len(pos)
            if n > M_PAD:
                overflow.append((core, nm, pos[M_PAD:]))
                pos = pos[:M_PAD]
                n = M_PAD
            X = np.zeros((d, M_PAD), np.float32)
            if n:
                X[:, :n] = emb[nm][t[pos]].T
            im[f"xt_{nm}"] = X
            im[f"w_{nm}"] = W[nm]
            r[nm] = (pos, n)
        in_maps.append(im)
        route.append(r)

    nc = _get_nc()
    res = run_bass_kernel_spmd(nc, in_maps, list(range(N_CORES)),
                               trace=bool(_profile))
    LAST_EXEC_NS = res.exec_time_ns
    LAST_RESULTS = res

    out = np.empty((B * S, D), np.float32)
    for core in range(N_CORES):
        lo = core * TOK_PER_CORE
        for nm in NAMES:
            pos, n = route[core][nm]
            if n:
                yt = res.results[core][f"yt_{nm}"]     # [D, M_PAD]
                out[lo + pos] = yt[:, :n].T
    # (astronomically unlikely) group overflow: compute the tail on host
    for core, nm, pos in overflow:
        lo = core * TOK_PER_CORE
        rows = emb[nm][tok_flat[lo + pos]]
        out[lo + pos] = rows @ W[nm] + bvec[nm]

    return out.reshape(B, S, D)


# revision 8
# speedup vs baseline: 1.2764x; 1.2764x over previous
"""Bass/Trainium2 kernel for nn_BespokeEmbedding (moe_routing).

Strategy (data-parallel over tokens across 8 NeuronCores):
  - Host computes per-token category codes (cat_table[token_ids]) and routes
    each core's 4096 tokens into 4 per-category groups (the "dispatch" step of
    the expert routing), gathering each group's embedding rows into a
    contraction-major [d_c, M_PAD] activation block per category.
  - Each core runs one Bass/Tile kernel: for every category, a dense
    float32r matmul  Y_c^T = W_c^T @ X_c^T  accumulated over 128-row K tiles
    in PSUM, fused bias-add on the Scalar engine, result streamed back as
    Y_c^T [D, M_PAD].
  - Host scatters rows back to token order (inverse of the dispatch) and
    returns the full [8, 4096, 1024] float32 output.

float32r keeps the full fp32 input bits and runs the PE at 1 cycle/row
(vs 4 for strict fp32); measured matmul relative error ~1.5e-4 at K=1536.
"""

import numpy as np

B, S, V, D = 8, 4096, 50257, 1024
CAT_DIMS = (1536, 1024, 512, 256)
NAMES = ("high", "mid", "low", "special")
N_CORES = 8
TOK_PER_CORE = (B * S) // N_CORES      # 4096
M_PAD = 1152                           # per-core per-category padded group size
CHUNKS = ((0, 512), (512, 512), (1024, 128))   # token chunks of M_PAD
N_DCOL = D // 128                      # 8

_CACHE = {}
LAST_EXEC_NS = None
LAST_RESULTS = None


def _build_bass():
    from contextlib import ExitStack
    import concourse.bacc as bacc
    import concourse.mybir as mybir
    import concourse.tile as tile

    nc = bacc.Bacc("TRN2", target_bir_lowering=False, debug=False,
                   num_devices=N_CORES)
    f16 = mybir.dt.float16
    f32 = mybir.dt.float32

    xt_d, w_d, yt_d = {}, {}, {}
    for nm, d in zip(NAMES, CAT_DIMS):
        xt_d[nm] = nc.dram_tensor(f"xt_{nm}", [d, M_PAD], f16, kind="ExternalInput")
        w_d[nm] = nc.dram_tensor(f"w_{nm}", [d, D], f16, kind="ExternalInput")
        yt_d[nm] = nc.dram_tensor(f"yt_{nm}", [D, M_PAD], f32, kind="ExternalOutput")
    # bias packed host-side as [128, 4*8]: column c*8+j holds b_c[j*128:(j+1)*128]
    bias_d = nc.dram_tensor("bias", [128, len(NAMES) * N_DCOL], f32,
                            kind="ExternalInput")

    with tile.TileContext(nc) as tc, ExitStack() as ctx:
        wpool = ctx.enter_context(tc.tile_pool(name="w", bufs=1))
        xpool = ctx.enter_context(tc.tile_pool(name="x", bufs=2))
        opool = ctx.enter_context(tc.tile_pool(name="o", bufs=2))
        bpool = ctx.enter_context(tc.tile_pool(name="b", bufs=1))
        ppool = ctx.enter_context(tc.tile_pool(name="p", bufs=4, space="PSUM"))

        bias_t = bpool.tile([128, len(NAMES) * N_DCOL], f32)
        nc.sync.dma_start(bias_t[:], bias_d.ap())

        # all four weight matrices stay resident in SBUF for the whole kernel:
        # total sum(d_c)/128 * 1024 fp16 = 52 KiB/partition
        w_t = {}
        for nm, d in zip(NAMES, CAT_DIMS):
            nk = d // 128
            w_t[nm] = wpool.tile([128, nk * D], f16, tag=f"w_{nm}", name=f"w_{nm}_sb")
            nc.sync.dma_start(
                w_t[nm][:].rearrange("p (k m) -> p k m", k=nk),
                w_d[nm].ap().rearrange("(k p) m -> p k m", p=128),
            )

        for ci, (nm, d) in enumerate(zip(NAMES, CAT_DIMS)):
            nk = d // 128
            for c0, n in CHUNKS:
                # activation slab [d, n] for this token chunk, K-major layout
                x_t = xpool.tile([128, 12 * 512], f16, tag="xslab")
                nc.sync.dma_start(
                    x_t[:, :nk * n].rearrange("p (k t) -> p k t", k=nk),
                    xt_d[nm].ap()[:, c0:c0 + n].rearrange("(k p) t -> p k t", p=128),
                )
                o_t = opool.tile([128, N_DCOL * 512], f32, tag="ostage")
                for j in range(N_DCOL):
                    ps = ppool.tile([128, 512], f32, tag="acc")
                    for k in range(nk):
                        nc.tensor.matmul(
                            ps[:, :n],
                            w_t[nm][:, k * D + j * 128: k * D + (j + 1) * 128],
                            x_t[:, k * n: (k + 1) * n],
                            start=(k == 0),
                            stop=(k == nk - 1),
                        )
                    nc.scalar.activation(
                        o_t[:, j * n: (j + 1) * n],
                        ps[:, :n],
                        bass_ident(),
                        bias=bias_t[:, ci * N_DCOL + j: ci * N_DCOL + j + 1],
                    )
                nc.sync.dma_start(
                    yt_d[nm].ap()[:, c0:c0 + n].rearrange("(j p) t -> p j t", p=128),
                    o_t[:, :N_DCOL * n].rearrange("p (j t) -> p j t", j=N_DCOL),
                )
    nc.compile()
    return nc


def bass_ident():
    import concourse.mybir as mybir
    return mybir.ActivationFunctionType.Identity


def _get_nc():
    if "nc" not in _CACHE:
        _CACHE["nc"] = _build_bass()
    return _CACHE["nc"]


def kernel(_profile=False, **inputs):
    global LAST_EXEC_NS, LAST_RESULTS
    from concourse.bass_utils import run_bass_kernel_spmd

    token_ids = np.asarray(inputs["token_ids"]).astype(np.int64)
    cat_table = np.asarray(inputs["cat_table"]).astype(np.int64)
    emb = {nm: np.ascontiguousarray(np.asarray(inputs[f"emb_{nm}"], dtype=np.float32))
           for nm in NAMES}
    W = {nm: np.ascontiguousarray(np.asarray(inputs[f"W_{nm}"], dtype=np.float32))
         for nm in NAMES}
    W16 = {nm: W[nm].astype(np.float16) for nm in NAMES}
    bvec = {nm: np.asarray(inputs[f"b_{nm}"], dtype=np.float32) for nm in NAMES}

    # bias packed as [128, 4*8]
    bias_packed = np.concatenate(
        [bvec[nm].reshape(N_DCOL, 128).T for nm in NAMES], axis=1
    )
    bias_packed = np.ascontiguousarray(bias_packed, dtype=np.float32)

    tok_flat = token_ids.reshape(-1)          # [32768]
    cats = cat_table[tok_flat]                # [32768]

    in_maps = []
    route = []      # per core: {nm: (positions, n_used)}
    overflow = []   # (core, nm, positions_beyond_cap)
    for core in range(N_CORES):
        lo = core * TOK_PER_CORE
        t = tok_flat[lo:lo + TOK_PER_CORE]
        c = cats[lo:lo + TOK_PER_CORE]
        im = {"bias": bias_packed}
        r = {}
        for ci, (nm, d) in enumerate(zip(NAMES, CAT_DIMS)):
            pos = np.nonzero(c == ci)[0]
            n = len(pos)
            if n > M_PAD:
                overflow.append((core, nm, pos[M_PAD:]))
                pos = pos[:M_PAD]
                n = M_PAD
            X = np.zeros((d, M_PAD), np.float16)
            if n:
                X[:, :n] = emb[nm][t[pos]].T
            im[f"xt_{nm}"] = X
            im[f"w_{nm}"] = W16[nm]
            r[nm] = (pos, n)
        in_maps.append(im)
        route.append(r)

    nc = _get_nc()
    res = run_bass_kernel_spmd(nc, in_maps, list(range(N_CORES)),
                               trace=bool(_profile))
    LAST_EXEC_NS = res.exec_time_ns
    LAST_RESULTS = res

    out = np.empty((B * S, D), np.float32)
    for core in range(N_CORES):
        lo = core * TOK_PER_CORE
        for nm in NAMES:
            pos, n = route[core][nm]
            if n:
                yt = res.results[core][f"yt_{nm}"]     # [D, M_PAD]
                out[lo + pos] = yt[:, :n].T
    # (astronomically unlikely) group overflow: compute the tail on host
    for core, nm, pos in overflow:
        lo = core * TOK_PER_CORE
        rows = emb[nm][tok_flat[lo + pos]]
        out[lo + pos] = rows @ W[nm] + bvec[nm]

    return out.reshape(B, S, D)


# revision 10
# speedup vs baseline: 1.3417x; 1.0511x over previous
"""Bass/Trainium2 kernel for nn_BespokeEmbedding (moe_routing).

Strategy (data-parallel over tokens across 8 NeuronCores):
  - Host computes per-token category codes (cat_table[token_ids]) and routes
    each core's 4096 tokens into 4 per-category groups (the "dispatch" step of
    the expert routing), gathering each group's embedding rows into a
    contraction-major [d_c, M_PAD] activation block per category.
  - Each core runs one Bass/Tile kernel: for every category, a dense
    float32r matmul  Y_c^T = W_c^T @ X_c^T  accumulated over 128-row K tiles
    in PSUM, fused bias-add on the Scalar engine, result streamed back as
    Y_c^T [D, M_PAD].
  - Host scatters rows back to token order (inverse of the dispatch) and
    returns the full [8, 4096, 1024] float32 output.

float32r keeps the full fp32 input bits and runs the PE at 1 cycle/row
(vs 4 for strict fp32); measured matmul relative error ~1.5e-4 at K=1536.
"""

import numpy as np

B, S, V, D = 8, 4096, 50257, 1024
CAT_DIMS = (1536, 1024, 512, 256)
NAMES = ("high", "mid", "low", "special")
N_CORES = 8
TOK_PER_CORE = (B * S) // N_CORES      # 4096
M_PAD = 1152                           # per-core per-category padded group size
CHUNKS = ((0, 512), (512, 512), (1024, 128))   # token chunks of M_PAD
N_DCOL = D // 128                      # 8

_CACHE = {}
LAST_EXEC_NS = None
LAST_RESULTS = None


def _build_bass():
    from contextlib import ExitStack
    import concourse.bacc as bacc
    import concourse.mybir as mybir
    import concourse.tile as tile

    nc = bacc.Bacc("TRN2", target_bir_lowering=False, debug=False,
                   num_devices=N_CORES)
    f16 = mybir.dt.float16
    f32 = mybir.dt.float32

    xt_d, w_d, yt_d = {}, {}, {}
    for nm, d in zip(NAMES, CAT_DIMS):
        xt_d[nm] = nc.dram_tensor(f"xt_{nm}", [d, M_PAD], f16, kind="ExternalInput")
        w_d[nm] = nc.dram_tensor(f"w_{nm}", [d, D], f16, kind="ExternalInput")
        yt_d[nm] = nc.dram_tensor(f"yt_{nm}", [D, M_PAD], f32, kind="ExternalOutput")
    # bias packed host-side as [128, 4*8]: column c*8+j holds b_c[j*128:(j+1)*128]
    bias_d = nc.dram_tensor("bias", [128, len(NAMES) * N_DCOL], f32,
                            kind="ExternalInput")

    with tile.TileContext(nc) as tc, ExitStack() as ctx:
        wpool = ctx.enter_context(tc.tile_pool(name="w", bufs=1))
        xpool = ctx.enter_context(tc.tile_pool(name="x", bufs=2))
        opool = ctx.enter_context(tc.tile_pool(name="o", bufs=3))
        bpool = ctx.enter_context(tc.tile_pool(name="b", bufs=1))
        ppool = ctx.enter_context(tc.tile_pool(name="p", bufs=2, space="PSUM"))

        bias_t = bpool.tile([128, len(NAMES) * N_DCOL], f32)
        nc.sync.dma_start(bias_t[:], bias_d.ap())

        # all four weight matrices stay resident in SBUF for the whole kernel:
        # total sum(d_c)/128 * 1024 fp16 = 52 KiB/partition
        w_t = {}
        for nm, d in zip(NAMES, CAT_DIMS):
            nk = d // 128
            w_t[nm] = wpool.tile([128, nk * D], f16, tag=f"w_{nm}", name=f"w_{nm}_sb")
            nc.sync.dma_start(
                w_t[nm][:].rearrange("p (k m) -> p k m", k=nk),
                w_d[nm].ap().rearrange("(k p) m -> p k m", p=128),
            )

        for ci, (nm, d) in enumerate(zip(NAMES, CAT_DIMS)):
            nk = d // 128
            # full activation block [d, M_PAD] for this category, K-major
            x_t = xpool.tile([128, 12 * M_PAD], f16, tag="xslab", name=f"x_{nm}")
            nc.sync.dma_start(
                x_t[:, :nk * M_PAD].rearrange("p (k t) -> p k t", k=nk),
                xt_d[nm].ap().rearrange("(k p) t -> p k t", p=128),
            )
            for j in range(N_DCOL):
                pss = [ppool.tile([128, 512], f32, tag=f"acc{q}", name=f"ps{q}")
                       for q in range(len(CHUNKS))]
                for k in range(nk):
                    # one stationary load of W[k-block, j-block] serves all
                    # three token chunks
                    for q, (c0, n) in enumerate(CHUNKS):
                        nc.tensor.matmul(
                            pss[q][:, :n],
                            w_t[nm][:, k * D + j * 128: k * D + (j + 1) * 128],
                            x_t[:, k * M_PAD + c0: k * M_PAD + c0 + n],
                            start=(k == 0),
                            stop=(k == nk - 1),
                        )
                o_t = opool.tile([128, M_PAD], f32, tag="ostage")
                for q, (c0, n) in enumerate(CHUNKS):
                    nc.scalar.activation(
                        o_t[:, c0:c0 + n],
                        pss[q][:, :n],
                        bass_ident(),
                        bias=bias_t[:, ci * N_DCOL + j: ci * N_DCOL + j + 1],
                    )
                nc.sync.dma_start(
                    yt_d[nm].ap()[j * 128:(j + 1) * 128, :],
                    o_t[:],
                )
    nc.compile()
    return nc


def bass_ident():
    import concourse.mybir as mybir
    return mybir.ActivationFunctionType.Identity


def _get_nc():
    if "nc" not in _CACHE:
        _CACHE["nc"] = _build_bass()
    return _CACHE["nc"]


def kernel(_profile=False, **inputs):
    global LAST_EXEC_NS, LAST_RESULTS
    from concourse.bass_utils import run_bass_kernel_spmd

    token_ids = np.asarray(inputs["token_ids"]).astype(np.int64)
    cat_table = np.asarray(inputs["cat_table"]).astype(np.int64)
    emb = {nm: np.ascontiguousarray(np.asarray(inputs[f"emb_{nm}"], dtype=np.float32))
           for nm in NAMES}
    W = {nm: np.ascontiguousarray(np.asarray(inputs[f"W_{nm}"], dtype=np.float32))
         for nm in NAMES}
    W16 = {nm: W[nm].astype(np.float16) for nm in NAMES}
    bvec = {nm: np.asarray(inputs[f"b_{nm}"], dtype=np.float32) for nm in NAMES}

    # bias packed as [128, 4*8]
    bias_packed = np.concatenate(
        [bvec[nm].reshape(N_DCOL, 128).T for nm in NAMES], axis=1
    )
    bias_packed = np.ascontiguousarray(bias_packed, dtype=np.float32)

    tok_flat = token_ids.reshape(-1)          # [32768]
    cats = cat_table[tok_flat]                # [32768]

    in_maps = []
    route = []      # per core: {nm: (positions, n_used)}
    overflow = []   # (core, nm, positions_beyond_cap)
    for core in range(N_CORES):
        lo = core * TOK_PER_CORE
        t = tok_flat[lo:lo + TOK_PER_CORE]
        c = cats[lo:lo + TOK_PER_CORE]
        im = {"bias": bias_packed}
        r = {}
        for ci, (nm, d) in enumerate(zip(NAMES, CAT_DIMS)):
            pos = np.nonzero(c == ci)[0]
            n = len(pos)
            if n > M_PAD:
                overflow.append((core, nm, pos[M_PAD:]))
                pos = pos[:M_PAD]
                n = M_PAD
            X = np.zeros((d, M_PAD), np.float16)
            if n:
                X[:, :n] = emb[nm][t[pos]].T
            im[f"xt_{nm}"] = X
            im[f"w_{nm}"] = W16[nm]
            r[nm] = (pos, n)
        in_maps.append(im)
        route.append(r)

    nc = _get_nc()
    res = run_bass_kernel_spmd(nc, in_maps, list(range(N_CORES)),
                               trace=bool(_profile))
    LAST_EXEC_NS = res.exec_time_ns
    LAST_RESULTS = res

    out = np.empty((B * S, D), np.float32)
    for core in range(N_CORES):
        lo = core * TOK_PER_CORE
        for nm in NAMES:
            pos, n = route[core][nm]
            if n:
                yt = res.results[core][f"yt_{nm}"]     # [D, M_PAD]
                out[lo + pos] = yt[:, :n].T
    # (astronomically unlikely) group overflow: compute the tail on host
    for core, nm, pos in overflow:
        lo = core * TOK_PER_CORE
        rows = emb[nm][tok_flat[lo + pos]]
        out[lo + pos] = rows @ W[nm] + bvec[nm]

    return out.reshape(B, S, D)
